# revision 26
# baseline (speedup 1.0000x reference)
"""Additive attention (Bahdanau) kernel for 8 Trainium2 NeuronCores.

Reference computation (per batch b):
    h   = enc_seq @ W_h.T                 [T, H]
    s   = dec_state @ W_s.T               [H]
    e_t = v . tanh(h_t + s)               [T]
    e   = where(mask==0, -1e9, e)
    a   = softmax(e)
    ctx = sum_t a_t * enc_seq[t]          [H]

Sharding: data-parallel over batch B=32 -> 4 batches per core, weights
replicated.

Main-matmul precision scheme (fp8 DoubleRow at 2x PE column throughput):
    W8 = fp8(W_h);  R = W_h - W8 (the quantization residual)
    psum = (16*W8)@x8 + (16*R)8@x8 = 16*h_approx      (all fp8e4 DoubleRow)
    tanh(psum * 1/16 + s) on ACT (the x16 pre-scale exists so the residual
    R survives fp8's limited subnormal range; 1/16 on the ACT undoes it)
  The residual pass removes the W-quantization error; remaining error is
  dominated by fp8(enc) in the scores (~1.05e-2 final rel err vs 2.1e-2
  without the residual pass, which is over the 2e-2 gate).

Mask handling (no per-chunk mask ops):
  ctx numerator uses host-premasked bf16 enc (enc*mask01); the denominator
  S = sum_t mask*p comes from one masked-sum DVE op per 4-chunk bank.

s = dec @ W_s.T is computed on the host (tiny) and shipped with v and the
mask chunks in ONE wide bf16 "aux" tile - tiny row-per-partition DMAs are
descriptor-issue-bound (~7us for a [128,4] transfer) so everything small
rides one [128, 4116] transfer instead.

Per-batch layout:
  e-rows for chunks 0-3 land on partitions {0,32,64,96} of one PSUM bank
  (matmul tile_position column offsets), chunks 4-7 on a second bank; exp
  runs on [97,512] (ACT timing is column-count-based, so the garbage rows
  between are free), a strided-partition DMA gathers the 4 rows into one
  DRAM row, and a stride-0 DMA broadcasts them back to all 128 partitions
  for the ctx accumulation (engines cannot read across partitions).

DMA queues: the SP queue carries the multi-MB enc loads + p-broadcasts;
the ACT queue carries the latency-critical small bounces (their exp/accum
dependencies are already satisfied when the ACT sequencer reaches them, so
they never head-of-line-block behind bulk traffic).

ctx accumulation: DVE scalar_tensor_tensor (p * x), one [128, 2048] op
per (bank, ht) with a single free-axis accumulate; dummy outputs rotate
through a deep pool so WAR release chains never stall the DVE. The last
batch flushes per-pair (not per-bank) to minimize the exposed tail, and
its outputs ship in a final dense [P, BL, OT] DMA (the host undoes the
transpose).
"""

import os
import sys
import numpy as np

sys.path.insert(0, "/opt/trn_rl_repo")

import ml_dtypes

B, T, H = 32, 4096, 512
NCORES = 8
BL = B // NCORES          # 4 batches per core
P = 128
KT = H // P               # 4 contraction tiles
OT = H // P               # 4 output tiles
TC = 512                  # t-chunk
NTC = T // TC             # 8 chunks per batch
RS = 16.0                 # fp8 residual scale (power of two; exact in fp8)
AUXW = KT + OT * BL + BL * 2 * TC + 4 * TC   # v | s | bank masks | last-batch pair masks

_CACHE = {}


def _build():
    import concourse.bass as bass
    import concourse.tile as tile
    from concourse import bacc, mybir
    from contextlib import ExitStack

    f32 = mybir.dt.float32
    bf16 = mybir.dt.bfloat16
    f8 = mybir.dt.float8e4
    ts = bass.ts
    Alu = mybir.AluOpType
    Act = mybir.ActivationFunctionType
    DR = mybir.MatmulPerfMode.DoubleRow

    nc = bacc.Bacc()

    enc8 = nc.declare_dram_parameter("enc8", [BL, P, KT, T], f8, isOutput=False)
    encm = nc.declare_dram_parameter("encm", [BL, P, KT, T], bf16, isOutput=False)
    auxd = nc.declare_dram_parameter("auxd", [P, AUXW], bf16, isOutput=False)
    w816 = nc.declare_dram_parameter("w816", [P, KT, H], f8, isOutput=False)
    r816 = nc.declare_dram_parameter("r816", [P, 2, H], f8, isOutput=False)
    out_e = nc.declare_dram_parameter("out", [P, BL, OT], f32, isOutput=True)

    with tile.TileContext(nc) as tc, ExitStack() as ctx:
        const = ctx.enter_context(tc.tile_pool(name="const", bufs=1))
        x8p = ctx.enter_context(tc.tile_pool(name="x8p", bufs=2))
        xbp = ctx.enter_context(tc.tile_pool(name="xbp", bufs=3))
        ttp = ctx.enter_context(tc.tile_pool(name="ttp", bufs=6))
        pxp = ctx.enter_context(tc.tile_pool(name="pxp", bufs=4))
        sap = ctx.enter_context(tc.tile_pool(name="sap", bufs=4))
        pbp = ctx.enter_context(tc.tile_pool(name="pbp", bufs=2))
        sbp = ctx.enter_context(tc.tile_pool(name="sbp", bufs=2))
        cap = ctx.enter_context(tc.tile_pool(name="cap", bufs=2))
        top = ctx.enter_context(tc.tile_pool(name="top", bufs=6))
        oup = ctx.enter_context(tc.tile_pool(name="oup", bufs=6))
        dramp = ctx.enter_context(tc.tile_pool(name="dramp", bufs=2, space="DRAM"))
        drsp = ctx.enter_context(tc.tile_pool(name="drsp", bufs=2, space="DRAM"))
        php = ctx.enter_context(tc.tile_pool(name="php", bufs=2, space="PSUM"))
        pep = ctx.enter_context(tc.tile_pool(name="pep", bufs=4, space="PSUM"))

        # ---- inputs: all on the SP queue in need-order (the ACT DGE queue
        # is ~20x slower - only tiny bounce DMAs go there); p-major host
        # layouts keep descriptors >= 1KB
        w8_sb = const.tile([P, KT, H], f8, tag="w8_sb")
        nc.sync.dma_start(w8_sb[:], w816[:])
        r8_sb = const.tile([P, 2, H], f8, tag="r8_sb")
        nc.sync.dma_start(r8_sb[:], r816[:])
        x8p_t = x8p.tile([P, KT, T], f8, tag="x8", name="x8_0")
        nc.sync.dma_start(x8p_t[:, :, : T // 2], enc8[0][:, :, : T // 2])
        aux = const.tile([P, AUXW], bf16, tag="aux")
        nc.sync.dma_start(aux[:], auxd[:, :])
        nc.sync.dma_start(x8p_t[:, :, T // 2 :], enc8[0][:, :, T // 2 :])
        xbp_t = xbp.tile([P, KT, T], bf16, tag="xb", name="xb_0")
        nc.sync.dma_start(xbp_t[:], encm[0])

        def v_ap(o):
            return aux[:, o : o + 1]

        def s_ap(o, b):
            i = KT + o * BL + b
            return aux[:, i : i + 1]

        def mask_ap(b, g):
            i = KT + OT * BL + (b * 2 + g) * TC
            return aux[0:97, i : i + TC]

        # ---- main pipeline ----
        def load_batch(b):
            x8 = x8p.tile([P, KT, T], f8, tag="x8", name=f"x8_{b}")
            nc.sync.dma_start(x8[:], enc8[b])
            xb = xbp.tile([P, KT, T], bf16, tag="xb", name=f"xb_{b}")
            nc.sync.dma_start(xb[:], encm[b])
            return x8, xb

        osball = const.tile([P, BL, OT], f32, tag="osball")
        nxt = (x8p_t, xbp_t)
        for b in range(BL):
            x8, xb = nxt
            if b + 1 < BL:
                nxt = load_batch(b + 1)
            if b == BL - 1:
                # first batches' outputs are long since final - flush them
                # while the last batch computes
                nc.sync.dma_start(out_e[:, 0 : BL - 2, :], osball[:, 0 : BL - 2, :])

            pd = dramp.tile([1, T], bf16, tag="pd")
            psd = drsp.tile([1, NTC], f32, tag="psd")
            pb = pbp.tile([P, T], bf16, tag="pb")
            nca = 4 if b == BL - 1 else 2
            ca = cap.tile([P, KT, nca], f32, tag=f"ca{nca}")
            pegs = []
            for g in range(2):
                peg = pep.tile([P, TC], f32, tag="pe", name=f"pe_b{b}g{g}")
                pegs.append(peg)
            last = b == BL - 1
            # scores: chunk-pairs (2m, 2m+1); pair m lives in bank m//2 at
            # rows {0,32} (m even) / {64,96} (m odd); the last batch gets one
            # bank per pair (rows {0,32}) so per-pair exp never WAR-blocks
            # another pair's e-dot matmuls
            for m in range(4):
                for o in range(OT):
                    ph = php.tile([P, 2, TC], f32, tag="ph")
                    # W8 on both k-pairs + fp8 residual on k-pair 0 only
                    # (half-K correction: rel err 0.0165 vs 0.0104 full,
                    # still well under the 2e-2 gate; saves 25% of main MMs)
                    parts = ((w8_sb, 0), (w8_sb, 1), (r8_sb, 0))
                    for wi, (wsb, kp) in enumerate(parts):
                        for j in range(2):
                            c = 2 * m + j
                            nc.tensor.matmul(
                                ph[:, j, :],
                                wsb[:, 2 * kp : 2 * kp + 2, ts(o, P)],
                                x8[:, 2 * kp : 2 * kp + 2, ts(c, TC)],
                                start=(wi == 0),
                                stop=(wi == len(parts) - 1),
                                perf_mode=DR,
                            )
                    tt = ttp.tile([P, 2, TC], bf16, tag="tt")
                    nc.scalar.activation(
                        tt[:], ph[:], Act.Tanh,
                        bias=s_ap(o, b), scale=1.0 / RS,
                    )
                    for j in range(2):
                        c = 2 * m + j
                        # last batch: alternate banks between pairs so the
                        # per-pair exp never WAR-blocks the next pair's MMs
                        q = 2 * (m // 2) + j if last else c % 4
                        bank = m % 2 if last else c // 4
                        nc.tensor.matmul(
                            pegs[bank][32 * q : 32 * q + 1, :],
                            v_ap(o),
                            tt[:, j, :],
                            start=(o == 0),
                            stop=(o == OT - 1),
                            tile_position=(0, 32 * q),
                            skip_group_check=True,
                        )
                # flush completed softmax units: whole banks normally;
                # per-pair units for the final batch so the exposed tail
                # after the last matmul is minimal
                units = []
                if not last:
                    if m == 1:
                        units = [(0, 97, 0, 4, 0, 0)]
                    elif m == 3:
                        units = [(0, 97, 4, 4, 1, 1)]
                else:
                    units = [(64 * (m // 2), 33, 2 * m, 2, m, m % 2)]
                for lo, cnt, c0, nch, ci, g in units:
                    rows = slice(lo, lo + cnt)
                    span = slice(c0 * TC, (c0 + nch) * TC)
                    pex = pxp.tile([P, TC], bf16, tag="pex")
                    nc.scalar.activation(pex[rows, :], pegs[g][rows, :], Act.Exp)
                    junk = pxp.tile([P, TC], bf16, tag="pex", name=f"junk{b}{m}")
                    sacc = sap.tile([P, 1], f32, tag="sacc")
                    if last:
                        mi = KT + OT * BL + BL * 2 * TC + ci * TC
                    else:
                        mi = KT + OT * BL + (b * 2 + c0 // 4) * TC
                    nc.vector.scalar_tensor_tensor(
                        out=junk[rows, :],
                        in0=pex[rows, :],
                        scalar=1.0,
                        in1=aux[rows, mi : mi + TC],
                        op0=Alu.mult,
                        op1=Alu.mult,
                        accum_out=sacc[rows, :],
                    )
                    nc.scalar.dma_start(
                        pd[0:1, span], pex[lo : lo + cnt : 32, :]
                    )
                    nc.scalar.dma_start(
                        psd[0:1, c0 : c0 + nch], sacc[lo : lo + cnt : 32, 0:1]
                    )
                    nc.sync.dma_start(
                        pb[:, span],
                        pd[0:1, span].to_broadcast((P, nch * TC)),
                    )
                    # ctx accumulation for the unit's chunks, one op per ht:
                    # ca[:, ht, ci] = sum_{t in unit} p[t] * xm[t]
                    for ht in range(KT):
                        to = top.tile([P, 4 * TC], bf16, tag="to")
                        nc.vector.scalar_tensor_tensor(
                            out=to[:, : nch * TC],
                            in0=xb[:, ht, span],
                            scalar=1.0,
                            in1=pb[:, span],
                            op0=Alu.mult,
                            op1=Alu.mult,
                            accum_out=ca[:, ht, ci : ci + 1],
                        )
            sbc = sbp.tile([P, NTC], f32, tag="sbc")
            nc.scalar.dma_start(sbc[:], psd[:].to_broadcast((P, NTC)))
            # normalize
            stot = oup.tile([P, 1], f32, tag="stot")
            nc.vector.tensor_reduce(
                stot[:], sbc[:], axis=mybir.AxisListType.X, op=Alu.add
            )
            rec = oup.tile([P, 1], f32, tag="rec")
            nc.vector.reciprocal(rec[:], stot[:])
            nc.vector.tensor_reduce(
                osball[:, b, :], ca[:], axis=mybir.AxisListType.X, op=Alu.add
            )
            nc.vector.tensor_mul(
                osball[:, b, :], osball[:, b, :], rec[:].to_broadcast((P, OT))
            )

        # remaining outputs at the very end
        nc.sync.dma_start(out_e[:, BL - 2 :, :], osball[:, BL - 2 :, :])

    nc.finalize()
    return nc


def _prep_in_maps(enc_seq, enc_mask, dec_state, W_h, W_s, v):
    bf = ml_dtypes.bfloat16
    f8 = ml_dtypes.float8_e4m3
    W8 = W_h.astype(f8).astype(np.float32)
    w816_f = ((W8 * RS).T).astype(f8)                  # [(k p), o]
    w816 = np.ascontiguousarray(w816_f.reshape(KT, P, H).transpose(1, 0, 2))
    # residual correction only on input dims 0:256 (half-K)
    r816_f = (((W_h - W8) * RS).T[: H // 2]).astype(f8)
    r816 = np.ascontiguousarray(r816_f.reshape(2, P, H).transpose(1, 0, 2))
    s_all = dec_state @ W_s.T                     # [B, H] f32 on host
    m01 = (enc_mask != 0).astype(np.float32)
    in_maps = []
    for ci in range(NCORES):
        sl = slice(ci * BL, (ci + 1) * BL)
        # p-major layout [BL, P, KT, T]: row p holds 16KB contiguous
        enc_t = enc_seq[sl].transpose(0, 2, 1)     # [BL, H, T]
        enc_pm = np.ascontiguousarray(
            enc_t.reshape(BL, KT, P, T).transpose(0, 2, 1, 3)
        )
        enc8 = enc_pm.astype(f8)
        encm = (enc_pm * m01[sl][:, None, None, :]).astype(bf)
        aux = np.zeros((P, AUXW), dtype=np.float32)
        aux[:, :KT] = v.reshape(KT, P).T
        # s[p, o, b] = s_all[b, o*128+p]
        s_c = s_all[sl].reshape(BL, OT, P)         # [b, o, p]
        aux[:, KT : KT + OT * BL] = s_c.transpose(2, 1, 0).reshape(P, OT * BL)
        # mask chunk c of batch b -> row 32*(c%4), col block (b*2 + c//4)
        mc = m01[sl].reshape(BL, 2, 4, TC)         # [b, g, q, i]
        mb = KT + OT * BL
        for q in range(4):
            aux[32 * q, mb : mb + BL * 2 * TC] = mc[:, :, q, :].reshape(
                BL * 2 * TC
            )
        # last batch's pair masks: chunk 2m+j at row 64*(m//2)+32*j, block m
        mp = mb + BL * 2 * TC
        ml = m01[sl][BL - 1].reshape(NTC, TC)
        for m in range(4):
            for j in range(2):
                aux[64 * (m // 2) + 32 * j, mp + m * TC : mp + (m + 1) * TC] = (
                    ml[2 * m + j]
                )
        in_maps.append(
            {
                "enc8": enc8,
                "encm": encm,
                "auxd": aux.astype(bf),
                "w816": w816,
                "r816": r816,
            }
        )
    return in_maps


def _run(inputs, trace=False):
    from concourse.bass_utils import run_bass_kernel_spmd

    if "nc" not in _CACHE:
        _CACHE["nc"] = _build()
    nc = _CACHE["nc"]
    in_maps = _prep_in_maps(**{k: np.asarray(v) for k, v in inputs.items()})
    res = run_bass_kernel_spmd(nc, in_maps, core_ids=list(range(NCORES)), trace=trace)
    # per-core out is [P, BL, OT]; batch-major layout is out[b, ht*128+p]
    out = np.concatenate(
        [
            np.transpose(res.results[c]["out"], (1, 2, 0)).reshape(BL, H)
            for c in range(NCORES)
        ],
        axis=0,
    )
    return out.astype(np.float32), res


def kernel(**inputs):
    out, _ = _run(inputs, trace=False)
    return out


# revision 27
# speedup vs baseline: 1.0282x; 1.0282x over previous
"""Additive attention (Bahdanau) kernel for 8 Trainium2 NeuronCores.

Reference computation (per batch b):
    h   = enc_seq @ W_h.T                 [T, H]
    s   = dec_state @ W_s.T               [H]
    e_t = v . tanh(h_t + s)               [T]
    e   = where(mask==0, -1e9, e)
    a   = softmax(e)
    ctx = sum_t a_t * enc_seq[t]          [H]

Sharding: data-parallel over batch B=32 -> 4 batches per core, weights
replicated.

Main-matmul precision scheme (fp8 DoubleRow at 2x PE column throughput):
    W8 = fp8(W_h);  R = W_h - W8 (the quantization residual)
    psum = (16*W8)@x8 + (16*R)8@x8 = 16*h_approx      (all fp8e4 DoubleRow)
    tanh(psum * 1/16 + s) on ACT (the x16 pre-scale exists so the residual
    R survives fp8's limited subnormal range; 1/16 on the ACT undoes it)
  The residual pass removes the W-quantization error; remaining error is
  dominated by fp8(enc) in the scores (~1.05e-2 final rel err vs 2.1e-2
  without the residual pass, which is over the 2e-2 gate).

Mask handling (no per-chunk mask ops):
  ctx numerator uses host-premasked bf16 enc (enc*mask01); the denominator
  S = sum_t mask*p comes from one masked-sum DVE op per 4-chunk bank.

s = dec @ W_s.T is computed on the host (tiny) and shipped with v and the
mask chunks in ONE wide bf16 "aux" tile - tiny row-per-partition DMAs are
descriptor-issue-bound (~7us for a [128,4] transfer) so everything small
rides one [128, 4116] transfer instead.

Per-batch layout:
  e-rows for chunks 0-3 land on partitions {0,32,64,96} of one PSUM bank
  (matmul tile_position column offsets), chunks 4-7 on a second bank; exp
  runs on [97,512] (ACT timing is column-count-based, so the garbage rows
  between are free), a strided-partition DMA gathers the 4 rows into one
  DRAM row, and a stride-0 DMA broadcasts them back to all 128 partitions
  for the ctx accumulation (engines cannot read across partitions).

DMA queues: the SP queue carries the multi-MB enc loads + p-broadcasts;
the ACT queue carries the latency-critical small bounces (their exp/accum
dependencies are already satisfied when the ACT sequencer reaches them, so
they never head-of-line-block behind bulk traffic).

ctx accumulation: DVE scalar_tensor_tensor (p * x), one [128, 2048] op
per (bank, ht) with a single free-axis accumulate; dummy outputs rotate
through a deep pool so WAR release chains never stall the DVE. The last
batch flushes per-pair (not per-bank) to minimize the exposed tail, and
its outputs ship in a final dense [P, BL, OT] DMA (the host undoes the
transpose).
"""

import os
import sys
import numpy as np

sys.path.insert(0, "/opt/trn_rl_repo")

import ml_dtypes

B, T, H = 32, 4096, 512
NCORES = 8
BL = B // NCORES          # 4 batches per core
P = 128
KT = H // P               # 4 contraction tiles
OT = H // P               # 4 output tiles
TC = 512                  # t-chunk
NTC = T // TC             # 8 chunks per batch
RS = 16.0                 # fp8 residual scale (power of two; exact in fp8)
AUXW = KT + OT * BL + BL * 2 * TC   # v | s | mask chunks

_CACHE = {}


def _build():
    import concourse.bass as bass
    import concourse.tile as tile
    from concourse import bacc, mybir
    from contextlib import ExitStack

    f32 = mybir.dt.float32
    bf16 = mybir.dt.bfloat16
    f8 = mybir.dt.float8e4
    ts = bass.ts
    Alu = mybir.AluOpType
    Act = mybir.ActivationFunctionType
    DR = mybir.MatmulPerfMode.DoubleRow

    nc = bacc.Bacc()

    enc8 = nc.declare_dram_parameter("enc8", [BL, P, KT, T], f8, isOutput=False)
    encm = nc.declare_dram_parameter("encm", [BL, P, KT, T], bf16, isOutput=False)
    auxd = nc.declare_dram_parameter("auxd", [P, AUXW], bf16, isOutput=False)
    w816 = nc.declare_dram_parameter("w816", [P, KT, H], f8, isOutput=False)
    r816 = nc.declare_dram_parameter("r816", [P, 2, H], f8, isOutput=False)
    out_e = nc.declare_dram_parameter("out", [P, BL, OT], f32, isOutput=True)

    with tile.TileContext(nc) as tc, ExitStack() as ctx:
        const = ctx.enter_context(tc.tile_pool(name="const", bufs=1))
        x8p = ctx.enter_context(tc.tile_pool(name="x8p", bufs=2))
        xbp = ctx.enter_context(tc.tile_pool(name="xbp", bufs=3))
        ttp = ctx.enter_context(tc.tile_pool(name="ttp", bufs=4))
        pxp = ctx.enter_context(tc.tile_pool(name="pxp", bufs=4))
        sap = ctx.enter_context(tc.tile_pool(name="sap", bufs=4))
        pbp = ctx.enter_context(tc.tile_pool(name="pbp", bufs=2))
        sbp = ctx.enter_context(tc.tile_pool(name="sbp", bufs=2))
        cap = ctx.enter_context(tc.tile_pool(name="cap", bufs=2))
        top = ctx.enter_context(tc.tile_pool(name="top", bufs=6))
        oup = ctx.enter_context(tc.tile_pool(name="oup", bufs=6))
        dramp = ctx.enter_context(tc.tile_pool(name="dramp", bufs=2, space="DRAM"))
        drsp = ctx.enter_context(tc.tile_pool(name="drsp", bufs=2, space="DRAM"))
        php = ctx.enter_context(tc.tile_pool(name="php", bufs=2, space="PSUM"))
        pep = ctx.enter_context(tc.tile_pool(name="pep", bufs=4, space="PSUM"))

        # ---- inputs: all on the SP queue in need-order (the ACT DGE queue
        # is ~20x slower - only tiny bounce DMAs go there); p-major host
        # layouts keep descriptors >= 1KB
        w8_sb = const.tile([P, KT, H], f8, tag="w8_sb")
        nc.sync.dma_start(w8_sb[:], w816[:])
        r8_sb = const.tile([P, 2, H], f8, tag="r8_sb")
        nc.sync.dma_start(r8_sb[:], r816[:])
        x8p_t = x8p.tile([P, KT, T], f8, tag="x8", name="x8_0")
        nc.sync.dma_start(x8p_t[:, :, : T // 2], enc8[0][:, :, : T // 2])
        aux = const.tile([P, AUXW], bf16, tag="aux")
        nc.sync.dma_start(aux[:], auxd[:, :])
        nc.sync.dma_start(x8p_t[:, :, T // 2 :], enc8[0][:, :, T // 2 :])
        xbp_t = xbp.tile([P, KT, T], bf16, tag="xb", name="xb_0")
        nc.sync.dma_start(xbp_t[:], encm[0])

        def v_ap(o):
            return aux[:, o : o + 1]

        def s_ap(o, b):
            i = KT + o * BL + b
            return aux[:, i : i + 1]

        def mask_ap(b, g):
            i = KT + OT * BL + (b * 2 + g) * TC
            return aux[0:97, i : i + TC]

        # ---- main pipeline ----
        def load_batch(b):
            x8 = x8p.tile([P, KT, T], f8, tag="x8", name=f"x8_{b}")
            nc.sync.dma_start(x8[:], enc8[b])
            xb = xbp.tile([P, KT, T], bf16, tag="xb", name=f"xb_{b}")
            nc.sync.dma_start(xb[:], encm[b])
            return x8, xb

        osball = const.tile([P, BL, OT], f32, tag="osball")
        nxt = (x8p_t, xbp_t)
        for b in range(BL):
            x8, xb = nxt
            if b + 1 < BL:
                nxt = load_batch(b + 1)
            if b == BL - 1:
                # first batches' outputs are long since final - flush them
                # while the last batch computes
                nc.sync.dma_start(out_e[:, 0 : BL - 2, :], osball[:, 0 : BL - 2, :])

            pd = dramp.tile([1, T], bf16, tag="pd")
            psd = drsp.tile([1, NTC], f32, tag="psd")
            pb = pbp.tile([P, T], bf16, tag="pb")
            nca = 4 if b == BL - 1 else 2
            ca = cap.tile([P, KT, nca], f32, tag=f"ca{nca}")
            pegs = []
            for g in range(2):
                peg = pep.tile([P, TC], f32, tag="pe", name=f"pe_b{b}g{g}")
                pegs.append(peg)
            last = b == BL - 1
            # scores: chunk-pairs (2m, 2m+1); pair m lives in bank m//2 at
            # rows {0,32} (m even) / {64,96} (m odd); the last batch gets one
            # bank per pair (rows {0,32}) so per-pair exp never WAR-blocks
            # another pair's e-dot matmuls
            for m in range(4):
                for o in range(OT):
                    ph = php.tile([P, 2, TC], f32, tag="ph")
                    # W8 on both k-pairs + fp8 residual on k-pair 0 only
                    # (half-K correction: rel err 0.0165 vs 0.0104 full,
                    # still well under the 2e-2 gate; saves 25% of main MMs)
                    parts = ((w8_sb, 0), (w8_sb, 1), (r8_sb, 0))
                    for wi, (wsb, kp) in enumerate(parts):
                        for j in range(2):
                            c = 2 * m + j
                            nc.tensor.matmul(
                                ph[:, j, :],
                                wsb[:, 2 * kp : 2 * kp + 2, ts(o, P)],
                                x8[:, 2 * kp : 2 * kp + 2, ts(c, TC)],
                                start=(wi == 0),
                                stop=(wi == len(parts) - 1),
                                perf_mode=DR,
                            )
                    tt = ttp.tile([P, 2, TC], bf16, tag="tt")
                    nc.scalar.activation(
                        tt[:], ph[:], Act.Tanh,
                        bias=s_ap(o, b), scale=1.0 / RS,
                    )
                    for j in range(2):
                        c = 2 * m + j
                        q = c % 4
                        nc.tensor.matmul(
                            pegs[c // 4][32 * q : 32 * q + 1, :],
                            v_ap(o),
                            tt[:, j, :],
                            start=(o == 0),
                            stop=(o == OT - 1),
                            tile_position=(0, 32 * q),
                            skip_group_check=True,
                        )
                # flush completed softmax units: whole banks normally;
                # per-pair units for the final batch so the exposed tail
                # after the last matmul is minimal
                units = []
                if not last:
                    if m == 1:
                        units = [(0, 97, 0, 4, 0)]
                    elif m == 3:
                        units = [(0, 97, 4, 4, 1)]
                else:
                    units = [(64 * (m % 2), 33, 2 * m, 2, m)]
                for lo, cnt, c0, nch, ci in units:
                    g = c0 // 4
                    rows = slice(lo, lo + cnt)
                    span = slice(c0 * TC, (c0 + nch) * TC)
                    pex = pxp.tile([P, TC], bf16, tag="pex")
                    nc.scalar.activation(pex[rows, :], pegs[g][rows, :], Act.Exp)
                    junk = pxp.tile([P, TC], bf16, tag="pex", name=f"junk{b}{m}")
                    sacc = sap.tile([P, 1], f32, tag="sacc")
                    mi = KT + OT * BL + (b * 2 + c0 // 4) * TC
                    nc.vector.scalar_tensor_tensor(
                        out=junk[rows, :],
                        in0=pex[rows, :],
                        scalar=1.0,
                        in1=aux[rows, mi : mi + TC],
                        op0=Alu.mult,
                        op1=Alu.mult,
                        accum_out=sacc[rows, :],
                    )
                    nc.scalar.dma_start(
                        pd[0:1, span], pex[lo : lo + cnt : 32, :]
                    )
                    nc.scalar.dma_start(
                        psd[0:1, c0 : c0 + nch], sacc[lo : lo + cnt : 32, 0:1]
                    )
                    nc.sync.dma_start(
                        pb[:, span],
                        pd[0:1, span].to_broadcast((P, nch * TC)),
                    )
                    # ctx accumulation for the unit's chunks, one op per ht:
                    # ca[:, ht, ci] = sum_{t in unit} p[t] * xm[t]
                    for ht in range(KT):
                        to = top.tile([P, 4 * TC], bf16, tag="to")
                        nc.vector.scalar_tensor_tensor(
                            out=to[:, : nch * TC],
                            in0=xb[:, ht, span],
                            scalar=1.0,
                            in1=pb[:, span],
                            op0=Alu.mult,
                            op1=Alu.mult,
                            accum_out=ca[:, ht, ci : ci + 1],
                        )
            sbc = sbp.tile([P, NTC], f32, tag="sbc")
            nc.scalar.dma_start(sbc[:], psd[:].to_broadcast((P, NTC)))
            # normalize
            stot = oup.tile([P, 1], f32, tag="stot")
            nc.vector.tensor_reduce(
                stot[:], sbc[:], axis=mybir.AxisListType.X, op=Alu.add
            )
            rec = oup.tile([P, 1], f32, tag="rec")
            nc.vector.reciprocal(rec[:], stot[:])
            nc.vector.tensor_reduce(
                osball[:, b, :], ca[:], axis=mybir.AxisListType.X, op=Alu.add
            )
            nc.vector.tensor_mul(
                osball[:, b, :], osball[:, b, :], rec[:].to_broadcast((P, OT))
            )

        # remaining outputs at the very end
        nc.sync.dma_start(out_e[:, BL - 2 :, :], osball[:, BL - 2 :, :])

    nc.finalize()
    return nc


def _prep_in_maps(enc_seq, enc_mask, dec_state, W_h, W_s, v):
    bf = ml_dtypes.bfloat16
    f8 = ml_dtypes.float8_e4m3
    W8 = W_h.astype(f8).astype(np.float32)
    w816_f = ((W8 * RS).T).astype(f8)                  # [(k p), o]
    w816 = np.ascontiguousarray(w816_f.reshape(KT, P, H).transpose(1, 0, 2))
    # residual correction only on input dims 0:256 (half-K)
    r816_f = (((W_h - W8) * RS).T[: H // 2]).astype(f8)
    r816 = np.ascontiguousarray(r816_f.reshape(2, P, H).transpose(1, 0, 2))
    s_all = dec_state @ W_s.T                     # [B, H] f32 on host
    m01 = (enc_mask != 0).astype(np.float32)
    in_maps = []
    for ci in range(NCORES):
        sl = slice(ci * BL, (ci + 1) * BL)
        # p-major layout [BL, P, KT, T]: row p holds 16KB contiguous
        enc_t = enc_seq[sl].transpose(0, 2, 1)     # [BL, H, T]
        enc_pm = np.ascontiguousarray(
            enc_t.reshape(BL, KT, P, T).transpose(0, 2, 1, 3)
        )
        enc8 = enc_pm.astype(f8)
        encm = (enc_pm * m01[sl][:, None, None, :]).astype(bf)
        aux = np.zeros((P, AUXW), dtype=np.float32)
        aux[:, :KT] = v.reshape(KT, P).T
        # s[p, o, b] = s_all[b, o*128+p]
        s_c = s_all[sl].reshape(BL, OT, P)         # [b, o, p]
        aux[:, KT : KT + OT * BL] = s_c.transpose(2, 1, 0).reshape(P, OT * BL)
        # mask chunk c of batch b -> row 32*(c%4), col block (b*2 + c//4)
        mc = m01[sl].reshape(BL, 2, 4, TC)         # [b, g, q, i]
        for q in range(4):
            aux[32 * q, KT + OT * BL :] = mc[:, :, q, :].reshape(BL * 2 * TC)
        in_maps.append(
            {
                "enc8": enc8,
                "encm": encm,
                "auxd": aux.astype(bf),
                "w816": w816,
                "r816": r816,
            }
        )
    return in_maps


def _run(inputs, trace=False):
    from concourse.bass_utils import run_bass_kernel_spmd

    if "nc" not in _CACHE:
        _CACHE["nc"] = _build()
    nc = _CACHE["nc"]
    in_maps = _prep_in_maps(**{k: np.asarray(v) for k, v in inputs.items()})
    res = run_bass_kernel_spmd(nc, in_maps, core_ids=list(range(NCORES)), trace=trace)
    # per-core out is [P, BL, OT]; batch-major layout is out[b, ht*128+p]
    out = np.concatenate(
        [
            np.transpose(res.results[c]["out"], (1, 2, 0)).reshape(BL, H)
            for c in range(NCORES)
        ],
        axis=0,
    )
    return out.astype(np.float32), res


def kernel(**inputs):
    out, _ = _run(inputs, trace=False)
    return out


# revision 28
# speedup vs baseline: 1.0424x; 1.0139x over previous
"""Additive attention (Bahdanau) kernel for 8 Trainium2 NeuronCores.

Reference computation (per batch b):
    h   = enc_seq @ W_h.T                 [T, H]
    s   = dec_state @ W_s.T               [H]
    e_t = v . tanh(h_t + s)               [T]
    e   = where(mask==0, -1e9, e)
    a   = softmax(e)
    ctx = sum_t a_t * enc_seq[t]          [H]

Sharding: data-parallel over batch B=32 -> 4 batches per core, weights
replicated.

Main-matmul precision scheme (fp8 DoubleRow at 2x PE column throughput):
    W8 = fp8(W_h);  R = W_h - W8 (the quantization residual)
    psum = (16*W8)@x8 + (16*R)8@x8 = 16*h_approx      (all fp8e4 DoubleRow)
    tanh(psum * 1/16 + s) on ACT (the x16 pre-scale exists so the residual
    R survives fp8's limited subnormal range; 1/16 on the ACT undoes it)
  The residual pass removes the W-quantization error; remaining error is
  dominated by fp8(enc) in the scores (~1.05e-2 final rel err vs 2.1e-2
  without the residual pass, which is over the 2e-2 gate).

Mask handling (no per-chunk mask ops):
  ctx numerator uses host-premasked bf16 enc (enc*mask01); the denominator
  S = sum_t mask*p comes from one masked-sum DVE op per 4-chunk bank.

s = dec @ W_s.T is computed on the host (tiny) and shipped with v and the
mask chunks in ONE wide bf16 "aux" tile - tiny row-per-partition DMAs are
descriptor-issue-bound (~7us for a [128,4] transfer) so everything small
rides one [128, 4116] transfer instead.

Per-batch layout:
  e-rows for chunks 0-3 land on partitions {0,32,64,96} of one PSUM bank
  (matmul tile_position column offsets), chunks 4-7 on a second bank; exp
  runs on [97,512] (ACT timing is column-count-based, so the garbage rows
  between are free), a strided-partition DMA gathers the 4 rows into one
  DRAM row, and a stride-0 DMA broadcasts them back to all 128 partitions
  for the ctx accumulation (engines cannot read across partitions).

DMA queues: the SP queue carries the multi-MB enc loads + p-broadcasts;
the ACT queue carries the latency-critical small bounces (their exp/accum
dependencies are already satisfied when the ACT sequencer reaches them, so
they never head-of-line-block behind bulk traffic).

ctx accumulation: DVE scalar_tensor_tensor (p * x), one [128, 2048] op
per (bank, ht) with a single free-axis accumulate; dummy outputs rotate
through a deep pool so WAR release chains never stall the DVE. The last
batch flushes per-pair (not per-bank) to minimize the exposed tail, and
its outputs ship in a final dense [P, BL, OT] DMA (the host undoes the
transpose).
"""

import os
import sys
import numpy as np

sys.path.insert(0, "/opt/trn_rl_repo")

import ml_dtypes

B, T, H = 32, 4096, 512
NCORES = 8
BL = B // NCORES          # 4 batches per core
P = 128
KT = H // P               # 4 contraction tiles
OT = H // P               # 4 output tiles
TC = 512                  # t-chunk
NTC = T // TC             # 8 chunks per batch
RS = 16.0                 # fp8 residual scale (power of two; exact in fp8)
AUXW = KT + OT * BL + BL * 2 * TC   # v | s | mask chunks

_CACHE = {}


def _build():
    import concourse.bass as bass
    import concourse.tile as tile
    from concourse import bacc, mybir
    from contextlib import ExitStack

    f32 = mybir.dt.float32
    bf16 = mybir.dt.bfloat16
    f8 = mybir.dt.float8e4
    ts = bass.ts
    Alu = mybir.AluOpType
    Act = mybir.ActivationFunctionType
    DR = mybir.MatmulPerfMode.DoubleRow

    nc = bacc.Bacc()

    enc8 = nc.declare_dram_parameter("enc8", [BL, P, KT, T], f8, isOutput=False)
    encm = nc.declare_dram_parameter("encm", [BL, P, KT, T], bf16, isOutput=False)
    auxd = nc.declare_dram_parameter("auxd", [P, AUXW], bf16, isOutput=False)
    w816 = nc.declare_dram_parameter("w816", [P, KT, H], f8, isOutput=False)
    r816 = nc.declare_dram_parameter("r816", [P, 2, H], f8, isOutput=False)
    out_e = nc.declare_dram_parameter("out", [P, BL, OT], f32, isOutput=True)

    with tile.TileContext(nc) as tc, ExitStack() as ctx:
        const = ctx.enter_context(tc.tile_pool(name="const", bufs=1))
        x8p = ctx.enter_context(tc.tile_pool(name="x8p", bufs=2))
        xbp = ctx.enter_context(tc.tile_pool(name="xbp", bufs=3))
        ttp = ctx.enter_context(tc.tile_pool(name="ttp", bufs=4))
        pxp = ctx.enter_context(tc.tile_pool(name="pxp", bufs=4))
        sap = ctx.enter_context(tc.tile_pool(name="sap", bufs=4))
        pbp = ctx.enter_context(tc.tile_pool(name="pbp", bufs=2))
        sbp = ctx.enter_context(tc.tile_pool(name="sbp", bufs=2))
        cap = ctx.enter_context(tc.tile_pool(name="cap", bufs=2))
        top = ctx.enter_context(tc.tile_pool(name="top", bufs=6))
        oup = ctx.enter_context(tc.tile_pool(name="oup", bufs=6))
        dramp = ctx.enter_context(tc.tile_pool(name="dramp", bufs=2, space="DRAM"))
        drsp = ctx.enter_context(tc.tile_pool(name="drsp", bufs=2, space="DRAM"))
        php = ctx.enter_context(tc.tile_pool(name="php", bufs=2, space="PSUM"))
        pep = ctx.enter_context(tc.tile_pool(name="pep", bufs=4, space="PSUM"))

        # ---- inputs: all on the SP queue in need-order (the ACT DGE queue
        # is ~20x slower - only tiny bounce DMAs go there); p-major host
        # layouts keep descriptors >= 1KB
        w8_sb = const.tile([P, KT, H], f8, tag="w8_sb")
        nc.sync.dma_start(w8_sb[:], w816[:])
        r8_sb = const.tile([P, 2, H], f8, tag="r8_sb")
        nc.sync.dma_start(r8_sb[:], r816[:])
        x8p_t = x8p.tile([P, KT, T], f8, tag="x8", name="x8_0")
        nc.sync.dma_start(x8p_t[:, :, : T // 4], enc8[0][:, :, : T // 4])
        nc.sync.dma_start(
            x8p_t[:, :, T // 4 : T // 2], enc8[0][:, :, T // 4 : T // 2]
        )
        aux = const.tile([P, AUXW], bf16, tag="aux")
        nc.sync.dma_start(aux[:], auxd[:, :])
        nc.sync.dma_start(x8p_t[:, :, T // 2 :], enc8[0][:, :, T // 2 :])
        xbp_t = xbp.tile([P, KT, T], bf16, tag="xb", name="xb_0")
        nc.sync.dma_start(xbp_t[:], encm[0])

        def v_ap(o):
            return aux[:, o : o + 1]

        def s_ap(o, b):
            i = KT + o * BL + b
            return aux[:, i : i + 1]

        def mask_ap(b, g):
            i = KT + OT * BL + (b * 2 + g) * TC
            return aux[0:97, i : i + TC]

        # ---- main pipeline ----
        def load_batch(b):
            x8 = x8p.tile([P, KT, T], f8, tag="x8", name=f"x8_{b}")
            nc.sync.dma_start(x8[:], enc8[b])
            xb = xbp.tile([P, KT, T], bf16, tag="xb", name=f"xb_{b}")
            nc.sync.dma_start(xb[:], encm[b])
            return x8, xb

        osball = const.tile([P, BL, OT], f32, tag="osball")
        nxt = (x8p_t, xbp_t)
        for b in range(BL):
            x8, xb = nxt
            if b + 1 < BL:
                nxt = load_batch(b + 1)
            if b == BL - 1:
                # first batches' outputs are long since final - flush them
                # while the last batch computes
                nc.sync.dma_start(out_e[:, 0 : BL - 2, :], osball[:, 0 : BL - 2, :])

            pd = dramp.tile([1, T], bf16, tag="pd")
            psd = drsp.tile([1, NTC], f32, tag="psd")
            pb = pbp.tile([P, T], bf16, tag="pb")
            nca = 4 if b == BL - 1 else 2
            ca = cap.tile([P, KT, nca], f32, tag=f"ca{nca}")
            pegs = []
            for g in range(2):
                peg = pep.tile([P, TC], f32, tag="pe", name=f"pe_b{b}g{g}")
                pegs.append(peg)
            last = b == BL - 1
            # scores: chunk-pairs (2m, 2m+1); pair m lives in bank m//2 at
            # rows {0,32} (m even) / {64,96} (m odd); the last batch gets one
            # bank per pair (rows {0,32}) so per-pair exp never WAR-blocks
            # another pair's e-dot matmuls
            for m in range(4):
                for o in range(OT):
                    ph = php.tile([P, 2, TC], f32, tag="ph")
                    # W8 on both k-pairs + fp8 residual on k-pair 0 only
                    # (half-K correction: rel err 0.0165 vs 0.0104 full,
                    # still well under the 2e-2 gate; saves 25% of main MMs)
                    parts = ((w8_sb, 0), (w8_sb, 1), (r8_sb, 0))
                    for wi, (wsb, kp) in enumerate(parts):
                        for j in range(2):
                            c = 2 * m + j
                            nc.tensor.matmul(
                                ph[:, j, :],
                                wsb[:, 2 * kp : 2 * kp + 2, ts(o, P)],
                                x8[:, 2 * kp : 2 * kp + 2, ts(c, TC)],
                                start=(wi == 0),
                                stop=(wi == len(parts) - 1),
                                perf_mode=DR,
                            )
                    tt = ttp.tile([P, 2, TC], bf16, tag="tt")
                    nc.scalar.activation(
                        tt[:], ph[:], Act.Tanh,
                        bias=s_ap(o, b), scale=1.0 / RS,
                    )
                    for j in range(2):
                        c = 2 * m + j
                        q = c % 4
                        nc.tensor.matmul(
                            pegs[c // 4][32 * q : 32 * q + 1, :],
                            v_ap(o),
                            tt[:, j, :],
                            start=(o == 0),
                            stop=(o == OT - 1),
                            tile_position=(0, 32 * q),
                            skip_group_check=True,
                        )
                # flush completed softmax units: whole banks normally;
                # per-pair units for the final batch so the exposed tail
                # after the last matmul is minimal
                units = []
                if not last:
                    if m == 1:
                        units = [(0, 97, 0, 4, 0)]
                    elif m == 3:
                        units = [(0, 97, 4, 4, 1)]
                else:
                    units = [(64 * (m % 2), 33, 2 * m, 2, m)]
                for lo, cnt, c0, nch, ci in units:
                    g = c0 // 4
                    rows = slice(lo, lo + cnt)
                    span = slice(c0 * TC, (c0 + nch) * TC)
                    pex = pxp.tile([P, TC], bf16, tag="pex")
                    nc.scalar.activation(pex[rows, :], pegs[g][rows, :], Act.Exp)
                    junk = pxp.tile([P, TC], bf16, tag="pex", name=f"junk{b}{m}")
                    sacc = sap.tile([P, 1], f32, tag="sacc")
                    mi = KT + OT * BL + (b * 2 + c0 // 4) * TC
                    nc.vector.scalar_tensor_tensor(
                        out=junk[rows, :],
                        in0=pex[rows, :],
                        scalar=1.0,
                        in1=aux[rows, mi : mi + TC],
                        op0=Alu.mult,
                        op1=Alu.mult,
                        accum_out=sacc[rows, :],
                    )
                    nc.scalar.dma_start(
                        pd[0:1, span], pex[lo : lo + cnt : 32, :]
                    )
                    nc.scalar.dma_start(
                        psd[0:1, c0 : c0 + nch], sacc[lo : lo + cnt : 32, 0:1]
                    )
                    nc.sync.dma_start(
                        pb[:, span],
                        pd[0:1, span].to_broadcast((P, nch * TC)),
                    )
                    # ctx accumulation for the unit's chunks, one op per ht:
                    # ca[:, ht, ci] = sum_{t in unit} p[t] * xm[t]
                    for ht in range(KT):
                        to = top.tile([P, 4 * TC], bf16, tag="to")
                        nc.vector.scalar_tensor_tensor(
                            out=to[:, : nch * TC],
                            in0=xb[:, ht, span],
                            scalar=1.0,
                            in1=pb[:, span],
                            op0=Alu.mult,
                            op1=Alu.mult,
                            accum_out=ca[:, ht, ci : ci + 1],
                        )
            sbc = sbp.tile([P, NTC], f32, tag="sbc")
            nc.scalar.dma_start(sbc[:], psd[:].to_broadcast((P, NTC)))
            # normalize
            stot = oup.tile([P, 1], f32, tag="stot")
            nc.vector.tensor_reduce(
                stot[:], sbc[:], axis=mybir.AxisListType.X, op=Alu.add
            )
            rec = oup.tile([P, 1], f32, tag="rec")
            nc.vector.reciprocal(rec[:], stot[:])
            nc.vector.tensor_reduce(
                osball[:, b, :], ca[:], axis=mybir.AxisListType.X, op=Alu.add
            )
            nc.vector.tensor_mul(
                osball[:, b, :], osball[:, b, :], rec[:].to_broadcast((P, OT))
            )

        # remaining outputs at the very end
        nc.sync.dma_start(out_e[:, BL - 2 :, :], osball[:, BL - 2 :, :])

    nc.finalize()
    return nc


def _prep_in_maps(enc_seq, enc_mask, dec_state, W_h, W_s, v):
    bf = ml_dtypes.bfloat16
    f8 = ml_dtypes.float8_e4m3
    W8 = W_h.astype(f8).astype(np.float32)
    w816_f = ((W8 * RS).T).astype(f8)                  # [(k p), o]
    w816 = np.ascontiguousarray(w816_f.reshape(KT, P, H).transpose(1, 0, 2))
    # residual correction only on input dims 0:256 (half-K)
    r816_f = (((W_h - W8) * RS).T[: H // 2]).astype(f8)
    r816 = np.ascontiguousarray(r816_f.reshape(2, P, H).transpose(1, 0, 2))
    s_all = dec_state @ W_s.T                     # [B, H] f32 on host
    m01 = (enc_mask != 0).astype(np.float32)
    in_maps = []
    for ci in range(NCORES):
        sl = slice(ci * BL, (ci + 1) * BL)
        # p-major layout [BL, P, KT, T]: row p holds 16KB contiguous
        enc_t = enc_seq[sl].transpose(0, 2, 1)     # [BL, H, T]
        enc_pm = np.ascontiguousarray(
            enc_t.reshape(BL, KT, P, T).transpose(0, 2, 1, 3)
        )
        enc8 = enc_pm.astype(f8)
        encm = (enc_pm * m01[sl][:, None, None, :]).astype(bf)
        aux = np.zeros((P, AUXW), dtype=np.float32)
        aux[:, :KT] = v.reshape(KT, P).T
        # s[p, o, b] = s_all[b, o*128+p]
        s_c = s_all[sl].reshape(BL, OT, P)         # [b, o, p]
        aux[:, KT : KT + OT * BL] = s_c.transpose(2, 1, 0).reshape(P, OT * BL)
        # mask chunk c of batch b -> row 32*(c%4), col block (b*2 + c//4)
        mc = m01[sl].reshape(BL, 2, 4, TC)         # [b, g, q, i]
        for q in range(4):
            aux[32 * q, KT + OT * BL :] = mc[:, :, q, :].reshape(BL * 2 * TC)
        in_maps.append(
            {
                "enc8": enc8,
                "encm": encm,
                "auxd": aux.astype(bf),
                "w816": w816,
                "r816": r816,
            }
        )
    return in_maps


def _run(inputs, trace=False):
    from concourse.bass_utils import run_bass_kernel_spmd

    if "nc" not in _CACHE:
        _CACHE["nc"] = _build()
    nc = _CACHE["nc"]
    in_maps = _prep_in_maps(**{k: np.asarray(v) for k, v in inputs.items()})
    res = run_bass_kernel_spmd(nc, in_maps, core_ids=list(range(NCORES)), trace=trace)
    # per-core out is [P, BL, OT]; batch-major layout is out[b, ht*128+p]
    out = np.concatenate(
        [
            np.transpose(res.results[c]["out"], (1, 2, 0)).reshape(BL, H)
            for c in range(NCORES)
        ],
        axis=0,
    )
    return out.astype(np.float32), res


def kernel(**inputs):
    out, _ = _run(inputs, trace=False)
    return out
